# revision 7
# baseline (speedup 1.0000x reference)
"""Trainium2 Bass kernel for nn_CrossAttentionBlock_44289702756632.

Math simplification (exact): the cross-attention attends a causal softmax over a
single time-repeated key, so every unmasked logit in a softmax row is identical
-> uniform weights -> att @ V == V (V is constant over the key-time axis).
Q / wq / bq never affect the output.  The block reduces to:

    x1  = LN1(x)
    yv  = cx @ (wkv[:, D:] @ wo) + (bkv[D:] @ wo + bo)     # host-precomputed
    x2  = x1 + yv[..., None, :]                            # broadcast over T
    x3  = LN2(x2)
    out = x3 + gelu(x3 @ w_fc + b_fc) @ w_pr + b_pr

Sharding: 1024 (B*A) independent slices, 128 per core across 8 cores.

Device layout per slice (T=128 partitions, D=256 free):
  LN via bn_stats/bn_aggr on DVE; yv broadcast-added via GPSIMD;
  x3 transposed on PE (2x 128x128), fc matmul with w_fc stationary producing
  hT directly ([4D, T] layout), gelu on ACT psum->SBUF, pr matmul with hT
  stationary producing [T, D], residual add on DVE.  Matmuls run as float32r
  (full-rate PE path, TF32-like precision, fp32 accumulate).
"""

import numpy as np

B, A, T, D = 8, 128, 128, 256
DFF = 4 * D
N_CORES = 8
S_TOTAL = B * A           # 1024 independent (b, a) slices
S_CORE = S_TOTAL // N_CORES
DG = 8                    # slices per DMA group
MG = 2                    # slices per matmul group (fc moving N = MG*128 >= 256)
N_DG = S_CORE // DG
EPS = 1e-5

_cache = {}


def _build(flags, act="Gelu"):
    """Build (nc, meta) for the given general-path flags."""
    import concourse.bass as bass
    import concourse.tile as tile
    from concourse import bacc, mybir
    from contextlib import ExitStack

    need_g1, need_g2, need_bfc, need_bpr = flags
    f32 = mybir.dt.float32
    f32r = mybir.dt.float32r
    AF = mybir.ActivationFunctionType
    OP = mybir.AluOpType

    nc = bacc.Bacc("TRN2", target_bir_lowering=False, debug=False)

    x_d = nc.dram_tensor("xg", [S_CORE * T, D], f32, kind="ExternalInput").ap()
    yv_d = nc.dram_tensor("yv", [S_CORE, D], f32, kind="ExternalInput").ap()
    wfc_d = nc.dram_tensor("wfc", [D, DFF], f32r, kind="ExternalInput").ap()
    wpr_d = nc.dram_tensor("wpr", [DFF, D], f32r, kind="ExternalInput").ap()
    id_d = nc.dram_tensor("ident", [T, T], f32, kind="ExternalInput").ap()
    if need_g1:
        g1_d = nc.dram_tensor("gam1", [D], f32, kind="ExternalInput").ap()
    if need_g2:
        g2_d = nc.dram_tensor("gam2", [D], f32, kind="ExternalInput").ap()
    if need_bfc:
        bfc_d = nc.dram_tensor("bfc", [DFF], f32r, kind="ExternalInput").ap()
    if need_bpr:
        bpr_d = nc.dram_tensor("bpr", [D], f32r, kind="ExternalInput").ap()
    out_d = nc.dram_tensor("out", [S_CORE * T, D], f32, kind="ExternalOutput").ap()

    def bcast(ap_2d, parts=128):
        # replicate a DRAM row-block across all partitions (DRE replication)
        return bass.AP(tensor=ap_2d.tensor, offset=ap_2d.offset,
                       ap=[[0, parts]] + [list(p) for p in ap_2d.ap])

    with tile.TileContext(nc) as tc, ExitStack() as ctx:
        consts = ctx.enter_context(tc.tile_pool(name="consts", bufs=1))
        xpool = ctx.enter_context(tc.tile_pool(name="xg", bufs=2))
        yvpool = ctx.enter_context(tc.tile_pool(name="yvb", bufs=2))
        spool = ctx.enter_context(tc.tile_pool(name="stats", bufs=2))
        x3pool = ctx.enter_context(tc.tile_pool(name="x3", bufs=2))
        xtpool = ctx.enter_context(tc.tile_pool(name="x3T", bufs=3))
        htpool = ctx.enter_context(tc.tile_pool(name="hT", bufs=2))
        opool = ctx.enter_context(tc.tile_pool(name="outg", bufs=2))
        trppool = ctx.enter_context(tc.tile_pool(name="ptr", bufs=2, space="PSUM"))
        htppool = ctx.enter_context(tc.tile_pool(name="phT", bufs=1, space="PSUM"))
        oppool = ctx.enter_context(tc.tile_pool(name="pout", bufs=2, space="PSUM"))

        # ---- constants ----
        wfc_sb = consts.tile([128, 2, DFF], f32r)   # [d%128, d//128, f]
        nc.sync.dma_start(out=wfc_sb, in_=wfc_d.rearrange("(k p) f -> p k f", k=2))
        wpr_sb = consts.tile([128, 8, D], f32r)     # [f%128, f//128, d]
        nc.sync.dma_start(out=wpr_sb, in_=wpr_d.rearrange("(k p) d -> p k d", k=8))
        id_sb = consts.tile([128, T], f32)
        nc.sync.dma_start(out=id_sb, in_=id_d)
        eps_sb = consts.tile([128, 1], f32)
        nc.vector.memset(eps_sb, EPS)
        if need_g1:
            g1_sb = consts.tile([128, D], f32)
            nc.gpsimd.dma_start(out=g1_sb, in_=bcast(g1_d[None, :]))
        if need_g2:
            g2_sb = consts.tile([128, D], f32)
            nc.gpsimd.dma_start(out=g2_sb, in_=bcast(g2_d[None, :]))
        if need_bfc or need_bpr:
            ones_sb = consts.tile([1, MG * T], f32r)
            nc.vector.memset(ones_sb, 1.0)
        if need_bfc:
            bfc_sb = consts.tile([1, DFF], f32r)
            nc.sync.dma_start(out=bfc_sb, in_=bfc_d[None, :])
        if need_bpr:
            bpr_sb = consts.tile([1, D], f32r)
            nc.sync.dma_start(out=bpr_sb, in_=bpr_d[None, :])


        for g in range(N_DG):
            rows = slice(g * DG * T, (g + 1) * DG * T)
            xg = xpool.tile([128, DG, D], f32)
            nc.sync.dma_start(out=xg, in_=x_d[rows, :].rearrange("(s t) d -> t s d", s=DG))
            yvb = yvpool.tile([128, DG, D], f32)
            nc.gpsimd.dma_start(out=yvb, in_=bcast(yv_d[g * DG:(g + 1) * DG, :]))

            # ---- LN1 ----
            st1 = spool.tile([128, DG, 6], f32)
            for s in range(DG):
                nc.vector.bn_stats(st1[:, s, :], xg[:, s, :])
            mv1 = spool.tile([128, DG, 2], f32)
            for s in range(DG):
                nc.vector.bn_aggr(mv1[:, s, :], st1[:, s, :])
            sd1 = spool.tile([128, DG], f32)
            nc.scalar.activation(sd1, mv1[:, :, 1], AF.Sqrt, bias=eps_sb)
            rs1 = spool.tile([128, DG], f32)
            nc.vector.reciprocal(rs1, sd1)
            for s in range(DG):
                nc.vector.tensor_scalar(
                    out=xg[:, s, :], in0=xg[:, s, :],
                    scalar1=mv1[:, s, 0:1], scalar2=rs1[:, s:s + 1],
                    op0=OP.subtract, op1=OP.mult)
                if need_g1:
                    nc.vector.tensor_mul(xg[:, s, :], xg[:, s, :], g1_sb)

            # ---- x2 = x1 + yv (broadcast over T) ----
            nc.gpsimd.tensor_add(xg, xg, yvb)

            # ---- LN2 ----
            st2 = spool.tile([128, DG, 6], f32)
            for s in range(DG):
                nc.vector.bn_stats(st2[:, s, :], xg[:, s, :])
            mv2 = spool.tile([128, DG, 2], f32)
            for s in range(DG):
                nc.vector.bn_aggr(mv2[:, s, :], st2[:, s, :])
            sd2 = spool.tile([128, DG], f32)
            nc.scalar.activation(sd2, mv2[:, :, 1], AF.Sqrt, bias=eps_sb)
            rs2 = spool.tile([128, DG], f32)
            nc.vector.reciprocal(rs2, sd2)
            x3 = x3pool.tile([128, DG, D], f32)
            for s in range(DG):
                nc.vector.tensor_scalar(
                    out=x3[:, s, :], in0=xg[:, s, :],
                    scalar1=mv2[:, s, 0:1], scalar2=rs2[:, s:s + 1],
                    op0=OP.subtract, op1=OP.mult)

            outg = opool.tile([128, DG, D], f32)
            for mg in range(DG // MG):
                # ---- transpose x3 -> x3T ([d, (gg, t)]) ----
                ptr = trppool.tile([128, 2, MG, T], f32)
                for gg in range(MG):
                    s = mg * MG + gg
                    for k in range(2):
                        nc.tensor.transpose(
                            ptr[:, k, gg, :], x3[:, s, k * 128:(k + 1) * 128], id_sb)
                x3T = xtpool.tile([128, 2, MG * T], f32r)
                nc.scalar.copy(x3T, ptr)

                # ---- fc: hT[m*128+p, (gg,t)] ----
                phT = htppool.tile([128, 8, MG * T], f32)
                for m in range(8):
                    ms = slice(m * 128, (m + 1) * 128)
                    if need_bfc:
                        nc.tensor.matmul(phT[:, m, :], bfc_sb[0:1, ms],
                                         ones_sb[0:1, :], start=True, stop=False)
                    for k in range(2):
                        nc.tensor.matmul(
                            phT[:, m, :], wfc_sb[:, k, ms], x3T[:, k, :],
                            start=(k == 0 and not need_bfc), stop=(k == 1))

                hT = htpool.tile([128, 8, MG * T], f32r)
                nc.scalar.activation(hT[:, 0:4, :], phT[:, 0:4, :], getattr(AF, act))
                nc.scalar.activation(hT[:, 4:8, :], phT[:, 4:8, :], getattr(AF, act))

                # ---- pr: out[t, d] ----
                pout = oppool.tile([128, MG, D], f32)
                for gg in range(MG):
                    ts = slice(gg * T, (gg + 1) * T)
                    if need_bpr:
                        nc.tensor.matmul(pout[:, gg, :], ones_sb[0:1, 0:T],
                                         bpr_sb[0:1, :], start=True, stop=False)
                    for k in range(8):
                        nc.tensor.matmul(
                            pout[:, gg, :], hT[:, k, ts], wpr_sb[:, k, :],
                            start=(k == 0 and not need_bpr), stop=(k == 7))

                for gg in range(MG):
                    s = mg * MG + gg
                    if need_g2:
                        nc.vector.tensor_mul(x3[:, s, :], x3[:, s, :], g2_sb)
                    nc.vector.tensor_add(outg[:, s, :], x3[:, s, :], pout[:, gg, :])

            nc.sync.dma_start(
                out=out_d[rows, :].rearrange("(s t) d -> t s d", s=DG), in_=outg)

    nc.compile()
    return nc


def _prepare(inputs):
    """Host-side preprocessing: fold the degenerate attention + biases."""
    x = np.asarray(inputs["x"], dtype=np.float32)
    cx = np.asarray(inputs["cx"], dtype=np.float32)
    wkv = np.asarray(inputs["wkv"], dtype=np.float32)
    bkv = np.asarray(inputs["bkv"], dtype=np.float32)
    wo = np.asarray(inputs["wo"], dtype=np.float32)
    bo = np.asarray(inputs["bo"], dtype=np.float32)
    w_fc = np.asarray(inputs["w_fc"], dtype=np.float32)
    b_fc = np.asarray(inputs["b_fc"], dtype=np.float32)
    w_pr = np.asarray(inputs["w_pr"], dtype=np.float32)
    b_pr = np.asarray(inputs["b_pr"], dtype=np.float32)
    ln1_w = np.asarray(inputs["ln1_w"], dtype=np.float32)
    ln1_b = np.asarray(inputs["ln1_b"], dtype=np.float32)
    ln2_w = np.asarray(inputs["ln2_w"], dtype=np.float32)
    ln2_b = np.asarray(inputs["ln2_b"], dtype=np.float32)

    # attention collapses to a per-(b,a) vector yv added to every time step
    wvo = wkv[:, D:] @ wo
    bvo = bkv[D:] @ wo + bo
    yv = cx.reshape(S_TOTAL, D) @ wvo + bvo
    yv = yv + ln1_b[None, :]                    # fold LN1 beta

    need_g1 = not np.all(ln1_w == 1.0)
    need_g2 = not np.all(ln2_w == 1.0)
    # fold LN2 beta into the fc bias and the output bias
    wfc_eff = (ln2_w[:, None] * w_fc) if need_g2 else w_fc
    bfc_eff = b_fc + ln2_b @ w_fc
    bpr_eff = b_pr + ln2_b
    need_bfc = not np.all(bfc_eff == 0.0)
    need_bpr = not np.all(bpr_eff == 0.0)

    flags = (need_g1, need_g2, need_bfc, need_bpr)
    x_flat = np.ascontiguousarray(x.reshape(S_TOTAL, T, D))

    in_maps = []
    for c in range(N_CORES):
        m = {
            "xg": np.ascontiguousarray(
                x_flat[c * S_CORE:(c + 1) * S_CORE].reshape(S_CORE * T, D)),
            "yv": np.ascontiguousarray(yv[c * S_CORE:(c + 1) * S_CORE]),
            "wfc": np.ascontiguousarray(wfc_eff),
            "wpr": w_pr,
            "ident": np.eye(T, dtype=np.float32),
        }
        if need_g1:
            m["gam1"] = ln1_w
        if need_g2:
            m["gam2"] = ln2_w
        if need_bfc:
            m["bfc"] = bfc_eff
        if need_bpr:
            m["bpr"] = bpr_eff
        in_maps.append(m)
    return flags, in_maps


def run(inputs, trace=False):
    from concourse.bass_utils import run_bass_kernel_spmd

    flags, in_maps = _prepare(inputs)
    if flags not in _cache:
        _cache[flags] = _build(flags)
    nc = _cache[flags]
    res = run_bass_kernel_spmd(nc, in_maps, list(range(N_CORES)), trace=trace)
    out = np.concatenate([res.results[c]["out"] for c in range(N_CORES)], axis=0)
    return out.reshape(B, A, T, D), res


def kernel(**inputs):
    out, _ = run(inputs, trace=False)
    return out


# revision 11
# speedup vs baseline: 1.4668x; 1.4668x over previous
"""Trainium2 Bass kernel for nn_CrossAttentionBlock_44289702756632.

Math simplification (exact): the cross-attention applies a causal softmax over a
single time-repeated key, so every unmasked logit in a softmax row is identical
-> uniform weights -> att @ V == V (V is constant over the key-time axis).
Q / wq / bq never affect the output.  The block reduces to:

    x1  = LN1(x)
    yv  = cx @ (wkv[:, D:] @ wo) + (bkv[D:] @ wo + bo)     # host-precomputed
    x2  = x1 + yv[..., None, :]                            # broadcast over T
    x3  = LN2(x2)
    out = x3 + gelu(x3 @ w_fc + b_fc) @ w_pr + b_pr

Sharding: 1024 (B*A) independent slices, 128 per core across 8 cores.

Device pipeline per slice (T=128 partitions, D=256 free):
  LN stats via bn_stats/bn_aggr (DVE), rstd via Newton-Raphson rsqrt (DVE,
  avoids ACT Sqrt<->Gelu table thrashing), LN1 apply on GPSIMD, yv
  broadcast-add on GPSIMD, LN2 apply on DVE.  x3 transposed on PE; the MLP
  runs in bf16 (fp32 accumulate in PSUM): fc with w_fc stationary producing
  hT directly in [4D, T] layout, gelu on ACT (psum->SBUF, casts to bf16),
  pr with hT stationary producing [T, D]; fp32 residual add on DVE.
"""

import numpy as np

B, A, T, D = 8, 128, 128, 256
DFF = 4 * D
N_CORES = 8
S_TOTAL = B * A           # 1024 independent (b, a) slices
S_CORE = S_TOTAL // N_CORES
DG = 8                    # slices per DMA group
MG = 2                    # slices per matmul group
N_DG = S_CORE // DG
EPS = 1e-5

_cache = {}


def _build(flags, act="Gelu"):
    import concourse.bass as bass
    import concourse.tile as tile
    from concourse import bacc, mybir
    from contextlib import ExitStack

    need_g1, need_g2, need_bfc, need_bpr = flags
    f32 = mybir.dt.float32
    bf16 = mybir.dt.bfloat16
    AF = mybir.ActivationFunctionType
    OP = mybir.AluOpType

    nc = bacc.Bacc("TRN2", target_bir_lowering=False, debug=False)

    x_d = nc.dram_tensor("xg", [S_CORE * T, D], f32, kind="ExternalInput").ap()
    yv_d = nc.dram_tensor("yv", [S_CORE, D], f32, kind="ExternalInput").ap()
    wfc_d = nc.dram_tensor("wfc", [D, DFF], bf16, kind="ExternalInput").ap()
    wpr_d = nc.dram_tensor("wpr", [DFF, D], bf16, kind="ExternalInput").ap()
    id_d = nc.dram_tensor("ident", [T, T], f32, kind="ExternalInput").ap()
    if need_g1:
        g1_d = nc.dram_tensor("gam1", [D], f32, kind="ExternalInput").ap()
    if need_g2:
        g2_d = nc.dram_tensor("gam2", [D], f32, kind="ExternalInput").ap()
    if need_bfc:
        bfc_d = nc.dram_tensor("bfc", [DFF], bf16, kind="ExternalInput").ap()
    if need_bpr:
        bpr_d = nc.dram_tensor("bpr", [D], bf16, kind="ExternalInput").ap()
    out_d = nc.dram_tensor("out", [S_CORE * T, D], f32, kind="ExternalOutput").ap()

    def bcast(ap_2d, parts=128):
        # replicate a DRAM row-block across all partitions (DRE replication)
        return bass.AP(tensor=ap_2d.tensor, offset=ap_2d.offset,
                       ap=[[0, parts]] + [list(p) for p in ap_2d.ap])

    with tile.TileContext(nc) as tc, ExitStack() as ctx:
        consts = ctx.enter_context(tc.tile_pool(name="consts", bufs=1))
        xpool = ctx.enter_context(tc.tile_pool(name="xg", bufs=3))
        yvpool = ctx.enter_context(tc.tile_pool(name="yvb", bufs=2))
        spool = ctx.enter_context(tc.tile_pool(name="stats", bufs=2))
        x3pool = ctx.enter_context(tc.tile_pool(name="x3", bufs=3))
        xtpool = ctx.enter_context(tc.tile_pool(name="x3T", bufs=3))
        htpool = ctx.enter_context(tc.tile_pool(name="hT", bufs=2))
        opool = ctx.enter_context(tc.tile_pool(name="outg", bufs=2))
        trppool = ctx.enter_context(tc.tile_pool(name="ptr", bufs=2, space="PSUM"))
        htppool = ctx.enter_context(tc.tile_pool(name="phT", bufs=1, space="PSUM"))
        oppool = ctx.enter_context(tc.tile_pool(name="pout", bufs=2, space="PSUM"))

        def nr_rsqrt(var_ap, tag):
            """rstd = 1/sqrt(var + EPS) on DVE via 3 Newton-Raphson steps."""
            v = spool.tile([128, DG], f32, tag="v" + tag)
            nc.vector.tensor_scalar_add(v, var_ap, float(EPS))
            y = spool.tile([128, DG], f32, tag="y" + tag)
            nc.vector.tensor_scalar(out=y, in0=v, scalar1=-0.5, scalar2=1.5,
                                    op0=OP.mult, op1=OP.add)
            w = spool.tile([128, DG], f32, tag="w" + tag)
            for _ in range(3):
                nc.vector.tensor_mul(w, y, y)
                nc.vector.tensor_mul(w, w, v)
                nc.vector.tensor_scalar(out=w, in0=w, scalar1=-0.5, scalar2=1.5,
                                        op0=OP.mult, op1=OP.add)
                nc.vector.tensor_mul(y, y, w)
            return y

        # ---- constants ----
        wfc_sb = consts.tile([128, 2, DFF], bf16)   # [d%128, d//128, f]
        nc.sync.dma_start(out=wfc_sb, in_=wfc_d.rearrange("(k p) f -> p k f", k=2))
        wpr_sb = consts.tile([128, 8, D], bf16)     # [f%128, f//128, d]
        nc.sync.dma_start(out=wpr_sb, in_=wpr_d.rearrange("(k p) d -> p k d", k=8))
        id_sb = consts.tile([128, T], f32)
        nc.sync.dma_start(out=id_sb, in_=id_d)
        if need_g1:
            g1_sb = consts.tile([128, D], f32)
            nc.gpsimd.dma_start(out=g1_sb, in_=bcast(g1_d[None, :]))
        if need_g2:
            g2_sb = consts.tile([128, D], f32)
            nc.gpsimd.dma_start(out=g2_sb, in_=bcast(g2_d[None, :]))
        if need_bfc or need_bpr:
            ones_sb = consts.tile([1, MG * T], bf16)
            nc.vector.memset(ones_sb, 1.0)
        if need_bfc:
            bfc_sb = consts.tile([1, DFF], bf16)
            nc.sync.dma_start(out=bfc_sb, in_=bfc_d[None, :])
        if need_bpr:
            bpr_sb = consts.tile([1, D], bf16)
            nc.sync.dma_start(out=bpr_sb, in_=bpr_d[None, :])

        for g in range(N_DG):
            rows = slice(g * DG * T, (g + 1) * DG * T)
            xg = xpool.tile([128, DG, D], f32)
            nc.sync.dma_start(out=xg, in_=x_d[rows, :].rearrange("(s t) d -> t s d", s=DG))
            yvb = yvpool.tile([128, DG, D], f32)
            nc.gpsimd.dma_start(out=yvb, in_=bcast(yv_d[g * DG:(g + 1) * DG, :]))

            # ---- LN1 (stats on DVE, apply on GPSIMD) ----
            st1 = spool.tile([128, DG, 6], f32)
            for s in range(DG):
                nc.vector.bn_stats(st1[:, s, :], xg[:, s, :])
            mv1 = spool.tile([128, DG, 2], f32)
            for s in range(DG):
                nc.vector.bn_aggr(mv1[:, s, :], st1[:, s, :])
            rs1 = nr_rsqrt(mv1[:, :, 1], "1")
            for s in range(DG):
                nc.vector.tensor_scalar(
                    out=xg[:, s, :], in0=xg[:, s, :],
                    scalar1=mv1[:, s, 0:1], scalar2=rs1[:, s:s + 1],
                    op0=OP.subtract, op1=OP.mult)
                if need_g1:
                    nc.vector.tensor_mul(xg[:, s, :], xg[:, s, :], g1_sb)

            # ---- x2 = x1 + yv (broadcast over T) ----
            nc.gpsimd.tensor_add(xg, xg, yvb)

            # ---- LN2 (stats + apply on DVE) ----
            st2 = spool.tile([128, DG, 6], f32)
            for s in range(DG):
                nc.vector.bn_stats(st2[:, s, :], xg[:, s, :])
            mv2 = spool.tile([128, DG, 2], f32)
            for s in range(DG):
                nc.vector.bn_aggr(mv2[:, s, :], st2[:, s, :])
            rs2 = nr_rsqrt(mv2[:, :, 1], "2")
            x3 = x3pool.tile([128, DG, D], f32)
            for s in range(DG):
                nc.vector.tensor_scalar(
                    out=x3[:, s, :], in0=xg[:, s, :],
                    scalar1=mv2[:, s, 0:1], scalar2=rs2[:, s:s + 1],
                    op0=OP.subtract, op1=OP.mult)

            outg = opool.tile([128, DG, D], f32)
            for mg in range(DG // MG):
                # ---- transpose x3 -> x3T ([d, (gg, t)]) ----
                ptr = trppool.tile([128, 2, MG, T], f32)
                for gg in range(MG):
                    s = mg * MG + gg
                    for k in range(2):
                        nc.tensor.transpose(
                            ptr[:, k, gg, :], x3[:, s, k * 128:(k + 1) * 128], id_sb)
                x3T = xtpool.tile([128, 2, MG * T], bf16)
                nc.scalar.copy(x3T, ptr)

                # ---- fc: hT[m*128+p, (gg,t)], in halves so gelu/pr overlap ----
                phT = [htppool.tile([128, 4, MG * T], f32, name="phT%d" % h, tag="phT%d" % h)
                       for h in range(2)]
                hT = [htpool.tile([128, 4, MG * T], bf16, name="hT%d" % h, tag="hT%d" % h)
                      for h in range(2)]
                for h in range(2):
                    for m in range(4):
                        ms = slice((4 * h + m) * 128, (4 * h + m + 1) * 128)
                        if need_bfc:
                            nc.tensor.matmul(phT[h][:, m, :], bfc_sb[0:1, ms],
                                             ones_sb[0:1, :], start=True, stop=False)
                        for k in range(2):
                            nc.tensor.matmul(
                                phT[h][:, m, :], wfc_sb[:, k, ms], x3T[:, k, :],
                                start=(k == 0 and not need_bfc), stop=(k == 1))
                    nc.scalar.activation(hT[h], phT[h], getattr(AF, act))

                # ---- pr: out[t, d] = h @ w_pr ----
                # one psum accumulation group spans both gg (per-element
                # has_written separates the regions within the bank)
                pout = oppool.tile([128, MG, D], f32)
                first = True
                for gg in range(MG):
                    ts = slice(gg * T, (gg + 1) * T)
                    if need_bpr:
                        nc.tensor.matmul(pout[:, gg, :], ones_sb[0:1, 0:T],
                                         bpr_sb[0:1, :], start=first, stop=False)
                        first = False
                    for k in range(8):
                        h, m = k // 4, k % 4
                        nc.tensor.matmul(
                            pout[:, gg, :], hT[h][:, m, ts], wpr_sb[:, k, :],
                            start=first, stop=(gg == MG - 1 and k == 7))
                        first = False

                for gg in range(MG):
                    s = mg * MG + gg
                    if need_g2:
                        nc.vector.tensor_mul(x3[:, s, :], x3[:, s, :], g2_sb)
                    nc.vector.tensor_add(outg[:, s, :], x3[:, s, :], pout[:, gg, :])

            nc.sync.dma_start(
                out=out_d[rows, :].rearrange("(s t) d -> t s d", s=DG), in_=outg)

    nc.compile()
    return nc


def _prepare(inputs):
    """Host-side preprocessing: fold the degenerate attention + biases."""
    import ml_dtypes

    x = np.asarray(inputs["x"], dtype=np.float32)
    cx = np.asarray(inputs["cx"], dtype=np.float32)
    wkv = np.asarray(inputs["wkv"], dtype=np.float32)
    bkv = np.asarray(inputs["bkv"], dtype=np.float32)
    wo = np.asarray(inputs["wo"], dtype=np.float32)
    bo = np.asarray(inputs["bo"], dtype=np.float32)
    w_fc = np.asarray(inputs["w_fc"], dtype=np.float32)
    b_fc = np.asarray(inputs["b_fc"], dtype=np.float32)
    w_pr = np.asarray(inputs["w_pr"], dtype=np.float32)
    b_pr = np.asarray(inputs["b_pr"], dtype=np.float32)
    ln1_w = np.asarray(inputs["ln1_w"], dtype=np.float32)
    ln1_b = np.asarray(inputs["ln1_b"], dtype=np.float32)
    ln2_w = np.asarray(inputs["ln2_w"], dtype=np.float32)
    ln2_b = np.asarray(inputs["ln2_b"], dtype=np.float32)

    # attention collapses to a per-(b,a) vector yv added to every time step
    wvo = wkv[:, D:] @ wo
    bvo = bkv[D:] @ wo + bo
    yv = cx.reshape(S_TOTAL, D) @ wvo + bvo
    yv = yv + ln1_b[None, :]                    # fold LN1 beta

    need_g1 = not np.all(ln1_w == 1.0)
    need_g2 = not np.all(ln2_w == 1.0)
    # fold LN2 beta into the fc bias and the output bias
    wfc_eff = (ln2_w[:, None] * w_fc) if need_g2 else w_fc
    bfc_eff = b_fc + ln2_b @ w_fc
    bpr_eff = b_pr + ln2_b
    need_bfc = not np.all(bfc_eff == 0.0)
    need_bpr = not np.all(bpr_eff == 0.0)

    flags = (need_g1, need_g2, need_bfc, need_bpr)
    x_flat = np.ascontiguousarray(x.reshape(S_TOTAL, T, D))
    bf = ml_dtypes.bfloat16

    in_maps = []
    for c in range(N_CORES):
        m = {
            "xg": np.ascontiguousarray(
                x_flat[c * S_CORE:(c + 1) * S_CORE].reshape(S_CORE * T, D)),
            "yv": np.ascontiguousarray(yv[c * S_CORE:(c + 1) * S_CORE]),
            "wfc": np.ascontiguousarray(wfc_eff.astype(bf)),
            "wpr": np.ascontiguousarray(w_pr.astype(bf)),
            "ident": np.eye(T, dtype=np.float32),
        }
        if need_g1:
            m["gam1"] = ln1_w
        if need_g2:
            m["gam2"] = ln2_w
        if need_bfc:
            m["bfc"] = bfc_eff.astype(bf)
        if need_bpr:
            m["bpr"] = bpr_eff.astype(bf)
        in_maps.append(m)
    return flags, in_maps


def run(inputs, trace=False):
    from concourse.bass_utils import run_bass_kernel_spmd

    flags, in_maps = _prepare(inputs)
    if flags not in _cache:
        _cache[flags] = _build(flags)
    nc = _cache[flags]
    res = run_bass_kernel_spmd(nc, in_maps, list(range(N_CORES)), trace=trace)
    out = np.concatenate([res.results[c]["out"] for c in range(N_CORES)], axis=0)
    return out.reshape(B, A, T, D), res


def kernel(**inputs):
    out, _ = run(inputs, trace=False)
    return out


# revision 13
# speedup vs baseline: 1.4879x; 1.0144x over previous
"""Trainium2 Bass kernel for nn_CrossAttentionBlock_44289702756632.

Math simplification (exact): the cross-attention applies a causal softmax over a
single time-repeated key, so every unmasked logit in a softmax row is identical
-> uniform weights -> att @ V == V (V is constant over the key-time axis).
Q / wq / bq never affect the output.  The block reduces to:

    x1  = LN1(x)
    yv  = cx @ (wkv[:, D:] @ wo) + (bkv[D:] @ wo + bo)     # host-precomputed
    x2  = x1 + yv[..., None, :]                            # broadcast over T
    x3  = LN2(x2)
    out = x3 + gelu(x3 @ w_fc + b_fc) @ w_pr + b_pr

Sharding: 1024 (B*A) independent slices, 128 per core across 8 cores.

Device pipeline per slice (T=128 partitions, D=256 free):
  LN stats via bn_stats/bn_aggr (DVE), rstd via Newton-Raphson rsqrt (DVE,
  avoids ACT Sqrt<->Gelu table thrashing), LN1 apply on GPSIMD, yv
  broadcast-add on GPSIMD, LN2 apply on DVE.  x3 transposed on PE; the MLP
  runs in bf16 (fp32 accumulate in PSUM): fc with w_fc stationary producing
  hT directly in [4D, T] layout, gelu on ACT (psum->SBUF, casts to bf16),
  pr with hT stationary producing [T, D]; fp32 residual add on DVE.
"""

import numpy as np

B, A, T, D = 8, 128, 128, 256
DFF = 4 * D
N_CORES = 8
S_TOTAL = B * A           # 1024 independent (b, a) slices
S_CORE = S_TOTAL // N_CORES
DG = 8                    # slices per DMA group
MG = 2                    # slices per matmul group
N_DG = S_CORE // DG
EPS = 1e-5

_cache = {}


def _build(flags, act="Gelu"):
    import concourse.bass as bass
    import concourse.tile as tile
    from concourse import bacc, mybir
    from contextlib import ExitStack

    need_g1, need_g2, need_bfc, need_bpr = flags
    f32 = mybir.dt.float32
    bf16 = mybir.dt.bfloat16
    AF = mybir.ActivationFunctionType
    OP = mybir.AluOpType

    nc = bacc.Bacc("TRN2", target_bir_lowering=False, debug=False)

    x_d = nc.dram_tensor("xg", [S_CORE * T, D], f32, kind="ExternalInput").ap()
    yv_d = nc.dram_tensor("yv", [S_CORE, D], f32, kind="ExternalInput").ap()
    wfc_d = nc.dram_tensor("wfc", [D, DFF], bf16, kind="ExternalInput").ap()
    wpr_d = nc.dram_tensor("wpr", [DFF, D], bf16, kind="ExternalInput").ap()
    id_d = nc.dram_tensor("ident", [T, T], f32, kind="ExternalInput").ap()
    if need_g1:
        g1_d = nc.dram_tensor("gam1", [D], f32, kind="ExternalInput").ap()
    if need_g2:
        g2_d = nc.dram_tensor("gam2", [D], f32, kind="ExternalInput").ap()
    if need_bfc:
        bfc_d = nc.dram_tensor("bfc", [DFF], bf16, kind="ExternalInput").ap()
    if need_bpr:
        bpr_d = nc.dram_tensor("bpr", [D], bf16, kind="ExternalInput").ap()
    out_d = nc.dram_tensor("out", [S_CORE * T, D], f32, kind="ExternalOutput").ap()

    def bcast(ap_2d, parts=128):
        # replicate a DRAM row-block across all partitions (DRE replication)
        return bass.AP(tensor=ap_2d.tensor, offset=ap_2d.offset,
                       ap=[[0, parts]] + [list(p) for p in ap_2d.ap])

    with tile.TileContext(nc) as tc, ExitStack() as ctx:
        consts = ctx.enter_context(tc.tile_pool(name="consts", bufs=1))
        xpool = ctx.enter_context(tc.tile_pool(name="xg", bufs=3))
        yvpool = ctx.enter_context(tc.tile_pool(name="yvb", bufs=2))
        spool = ctx.enter_context(tc.tile_pool(name="stats", bufs=2))
        x3pool = ctx.enter_context(tc.tile_pool(name="x3", bufs=3))
        xtpool = ctx.enter_context(tc.tile_pool(name="x3T", bufs=3))
        htpool = ctx.enter_context(tc.tile_pool(name="hT", bufs=2))
        opool = ctx.enter_context(tc.tile_pool(name="outg", bufs=2))
        trppool = ctx.enter_context(tc.tile_pool(name="ptr", bufs=2, space="PSUM"))
        htppool = ctx.enter_context(tc.tile_pool(name="phT", bufs=1, space="PSUM"))
        oppool = ctx.enter_context(tc.tile_pool(name="pout", bufs=2, space="PSUM"))

        def nr_rsqrt(var_ap, tag, iters):
            """rstd = 1/sqrt(var + EPS) on DVE, Newton-Raphson from 1.5-0.5v."""
            v = spool.tile([128, DG], f32, tag="v" + tag)
            nc.vector.tensor_scalar_add(v, var_ap, float(EPS))
            y = spool.tile([128, DG], f32, tag="y" + tag)
            nc.vector.tensor_scalar(out=y, in0=v, scalar1=-0.5, scalar2=1.5,
                                    op0=OP.mult, op1=OP.add)
            w = spool.tile([128, DG], f32, tag="w" + tag)
            for _ in range(iters):
                nc.vector.tensor_mul(w, y, y)
                nc.vector.tensor_mul(w, w, v)
                nc.vector.tensor_scalar(out=w, in0=w, scalar1=-0.5, scalar2=1.5,
                                        op0=OP.mult, op1=OP.add)
                nc.vector.tensor_mul(y, y, w)
            return y

        # ---- constants ----
        wfc_sb = consts.tile([128, 2, DFF], bf16)   # [d%128, d//128, f]
        nc.sync.dma_start(out=wfc_sb, in_=wfc_d.rearrange("(k p) f -> p k f", k=2))
        wpr_sb = consts.tile([128, 8, D], bf16)     # [f%128, f//128, d]
        nc.sync.dma_start(out=wpr_sb, in_=wpr_d.rearrange("(k p) d -> p k d", k=8))
        id_sb = consts.tile([128, T], f32)
        nc.sync.dma_start(out=id_sb, in_=id_d)
        if need_g1:
            g1_sb = consts.tile([128, D], f32)
            nc.gpsimd.dma_start(out=g1_sb, in_=bcast(g1_d[None, :]))
        if need_g2:
            g2_sb = consts.tile([128, D], f32)
            nc.gpsimd.dma_start(out=g2_sb, in_=bcast(g2_d[None, :]))
        if need_bfc or need_bpr:
            ones_sb = consts.tile([1, MG * T], bf16)
            nc.vector.memset(ones_sb, 1.0)
        if need_bfc:
            bfc_sb = consts.tile([1, DFF], bf16)
            nc.sync.dma_start(out=bfc_sb, in_=bfc_d[None, :])
        if need_bpr:
            bpr_sb = consts.tile([1, D], bf16)
            nc.sync.dma_start(out=bpr_sb, in_=bpr_d[None, :])

        for g in range(N_DG):
            rows = slice(g * DG * T, (g + 1) * DG * T)
            xg = xpool.tile([128, DG, D], f32)
            nc.sync.dma_start(out=xg, in_=x_d[rows, :].rearrange("(s t) d -> t s d", s=DG))
            yvb = yvpool.tile([128, DG, D], f32)
            nc.gpsimd.dma_start(out=yvb, in_=bcast(yv_d[g * DG:(g + 1) * DG, :]))

            # ---- LN1 (stats on DVE, apply on GPSIMD) ----
            st1 = spool.tile([128, DG, 6], f32)
            for s in range(DG):
                nc.vector.bn_stats(st1[:, s, :], xg[:, s, :])
            mv1 = spool.tile([128, DG, 2], f32)
            for s in range(DG):
                nc.vector.bn_aggr(mv1[:, s, :], st1[:, s, :])
            rs1 = nr_rsqrt(mv1[:, :, 1], "1", 3)
            for s in range(DG):
                nc.vector.tensor_scalar(
                    out=xg[:, s, :], in0=xg[:, s, :],
                    scalar1=mv1[:, s, 0:1], scalar2=rs1[:, s:s + 1],
                    op0=OP.subtract, op1=OP.mult)
                if need_g1:
                    nc.vector.tensor_mul(xg[:, s, :], xg[:, s, :], g1_sb)

            # ---- x2 = x1 + yv (broadcast over T) ----
            nc.gpsimd.tensor_add(xg, xg, yvb)

            # ---- LN2 (stats + apply on DVE) ----
            st2 = spool.tile([128, DG, 6], f32)
            for s in range(DG):
                nc.vector.bn_stats(st2[:, s, :], xg[:, s, :])
            mv2 = spool.tile([128, DG, 2], f32)
            for s in range(DG):
                nc.vector.bn_aggr(mv2[:, s, :], st2[:, s, :])
            rs2 = nr_rsqrt(mv2[:, :, 1], "2", 2)
            x3 = x3pool.tile([128, DG, D], f32)
            for s in range(DG):
                nc.vector.tensor_scalar(
                    out=x3[:, s, :], in0=xg[:, s, :],
                    scalar1=mv2[:, s, 0:1], scalar2=rs2[:, s:s + 1],
                    op0=OP.subtract, op1=OP.mult)

            outg = opool.tile([128, DG, D], f32)
            for mg in range(DG // MG):
                # ---- transpose x3 -> x3T ([d, (gg, t)]) ----
                ptr = trppool.tile([128, 2, MG, T], f32)
                for gg in range(MG):
                    s = mg * MG + gg
                    for k in range(2):
                        nc.tensor.transpose(
                            ptr[:, k, gg, :], x3[:, s, k * 128:(k + 1) * 128], id_sb)
                x3T = xtpool.tile([128, 2, MG * T], bf16)
                nc.scalar.copy(x3T, ptr)

                # ---- fc: hT[m*128+p, (gg,t)], in halves so gelu/pr overlap ----
                phT = [htppool.tile([128, 4, MG * T], f32, name="phT%d" % h, tag="phT%d" % h)
                       for h in range(2)]
                hT = [htpool.tile([128, 4, MG * T], bf16, name="hT%d" % h, tag="hT%d" % h)
                      for h in range(2)]
                for h in range(2):
                    for m in range(4):
                        ms = slice((4 * h + m) * 128, (4 * h + m + 1) * 128)
                        if need_bfc:
                            nc.tensor.matmul(phT[h][:, m, :], bfc_sb[0:1, ms],
                                             ones_sb[0:1, :], start=True, stop=False)
                        for k in range(2):
                            nc.tensor.matmul(
                                phT[h][:, m, :], wfc_sb[:, k, ms], x3T[:, k, :],
                                start=(k == 0 and not need_bfc), stop=(k == 1))
                    nc.scalar.activation(hT[h], phT[h], getattr(AF, act))

                # ---- pr: out[t, d] = h @ w_pr ----
                # one psum accumulation group spans both gg (per-element
                # has_written separates the regions within the bank)
                pout = oppool.tile([128, MG, D], f32)
                first = True
                for gg in range(MG):
                    ts = slice(gg * T, (gg + 1) * T)
                    if need_bpr:
                        nc.tensor.matmul(pout[:, gg, :], ones_sb[0:1, 0:T],
                                         bpr_sb[0:1, :], start=first, stop=False)
                        first = False
                    for k in range(8):
                        h, m = k // 4, k % 4
                        nc.tensor.matmul(
                            pout[:, gg, :], hT[h][:, m, ts], wpr_sb[:, k, :],
                            start=first, stop=(gg == MG - 1 and k == 7))
                        first = False

                for gg in range(MG):
                    s = mg * MG + gg
                    if need_g2:
                        nc.vector.tensor_mul(x3[:, s, :], x3[:, s, :], g2_sb)
                    nc.vector.tensor_add(outg[:, s, :], x3[:, s, :], pout[:, gg, :])

            nc.sync.dma_start(
                out=out_d[rows, :].rearrange("(s t) d -> t s d", s=DG), in_=outg)

    nc.compile()
    return nc


def _prepare(inputs):
    """Host-side preprocessing: fold the degenerate attention + biases."""
    import ml_dtypes

    x = np.asarray(inputs["x"], dtype=np.float32)
    cx = np.asarray(inputs["cx"], dtype=np.float32)
    wkv = np.asarray(inputs["wkv"], dtype=np.float32)
    bkv = np.asarray(inputs["bkv"], dtype=np.float32)
    wo = np.asarray(inputs["wo"], dtype=np.float32)
    bo = np.asarray(inputs["bo"], dtype=np.float32)
    w_fc = np.asarray(inputs["w_fc"], dtype=np.float32)
    b_fc = np.asarray(inputs["b_fc"], dtype=np.float32)
    w_pr = np.asarray(inputs["w_pr"], dtype=np.float32)
    b_pr = np.asarray(inputs["b_pr"], dtype=np.float32)
    ln1_w = np.asarray(inputs["ln1_w"], dtype=np.float32)
    ln1_b = np.asarray(inputs["ln1_b"], dtype=np.float32)
    ln2_w = np.asarray(inputs["ln2_w"], dtype=np.float32)
    ln2_b = np.asarray(inputs["ln2_b"], dtype=np.float32)

    # attention collapses to a per-(b,a) vector yv added to every time step
    wvo = wkv[:, D:] @ wo
    bvo = bkv[D:] @ wo + bo
    yv = cx.reshape(S_TOTAL, D) @ wvo + bvo
    yv = yv + ln1_b[None, :]                    # fold LN1 beta

    need_g1 = not np.all(ln1_w == 1.0)
    need_g2 = not np.all(ln2_w == 1.0)
    # fold LN2 beta into the fc bias and the output bias
    wfc_eff = (ln2_w[:, None] * w_fc) if need_g2 else w_fc
    bfc_eff = b_fc + ln2_b @ w_fc
    bpr_eff = b_pr + ln2_b
    need_bfc = not np.all(bfc_eff == 0.0)
    need_bpr = not np.all(bpr_eff == 0.0)

    flags = (need_g1, need_g2, need_bfc, need_bpr)
    x_flat = np.ascontiguousarray(x.reshape(S_TOTAL, T, D))
    bf = ml_dtypes.bfloat16

    in_maps = []
    for c in range(N_CORES):
        m = {
            "xg": np.ascontiguousarray(
                x_flat[c * S_CORE:(c + 1) * S_CORE].reshape(S_CORE * T, D)),
            "yv": np.ascontiguousarray(yv[c * S_CORE:(c + 1) * S_CORE]),
            "wfc": np.ascontiguousarray(wfc_eff.astype(bf)),
            "wpr": np.ascontiguousarray(w_pr.astype(bf)),
            "ident": np.eye(T, dtype=np.float32),
        }
        if need_g1:
            m["gam1"] = ln1_w
        if need_g2:
            m["gam2"] = ln2_w
        if need_bfc:
            m["bfc"] = bfc_eff.astype(bf)
        if need_bpr:
            m["bpr"] = bpr_eff.astype(bf)
        in_maps.append(m)
    return flags, in_maps


def run(inputs, trace=False):
    from concourse.bass_utils import run_bass_kernel_spmd

    flags, in_maps = _prepare(inputs)
    if flags not in _cache:
        _cache[flags] = _build(flags)
    nc = _cache[flags]
    res = run_bass_kernel_spmd(nc, in_maps, list(range(N_CORES)), trace=trace)
    out = np.concatenate([res.results[c]["out"] for c in range(N_CORES)], axis=0)
    return out.reshape(B, A, T, D), res


def kernel(**inputs):
    out, _ = run(inputs, trace=False)
    return out
